# revision 4
# baseline (speedup 1.0000x reference)
"""AConnect forward kernel for one TRN2 chip (8 NeuronCores).

Computes Z[b] = X[b] @ (W * Werr[loc_id[b]]) + Berr[loc_id[b]] * bias
for B=128, IN=OUT=1024, POOL=200.

Strategy (data-parallel over the OUT dim, dedup over the pool ids):
  - Host: dedup loc_id into n_u unique pool entries (~95 of 128 draws),
    fold W into the gathered pool (bf16), sort samples by group, and pack
    per-core weight slabs so every Werr byte is read exactly once
    chip-wide. Each core owns a 128-column slice of OUT.
  - Device: for each "quad" of 4 groups, one 1 MiB DMA streams the packed
    weights; TensorE runs 8 k-tile matmuls (stationary = X^T tiles,
    moving = packed weights, N=512) plus one rank-4 indicator matmul that
    adds the per-group bias row; DVE extracts each group's 128-column
    block of PSUM into the output tile.
"""

import os
import sys
import types

import numpy as np

if "/opt/trn_rl_repo" not in sys.path:
    sys.path.insert(0, "/opt/trn_rl_repo")

import ml_dtypes

BF16 = ml_dtypes.bfloat16

BATCH, IN, OUT, POOL = 128, 1024, 1024, 200
N_CORES = 8
OSH = OUT // N_CORES  # 128 output columns per core
KT = IN // 128        # 8 k-tiles


def _install_ntff_hook():
    """Make run_bass_kernel_spmd(trace=True) work under axon: the glue
    module antenv.axon_hooks is absent from this image, so inject it."""
    if "antenv.axon_hooks" in sys.modules:
        return
    try:
        from trn_agent_boot.trn_boot import _ntff_profile_via_ctypes

        hook = _ntff_profile_via_ctypes("/opt/axon/libaxon_pjrt.so")
    except Exception:
        hook = None
    mod = types.ModuleType("antenv.axon_hooks")
    mod.get_axon_ntff_profile_hook = lambda: hook
    mod.set_axon_ntff_profile_hook = lambda h: None
    sys.modules["antenv.axon_hooks"] = mod


_NC_CACHE: dict = {}
LAST_EXEC_TIME_NS = None


def _build_graph(n_q, bounds):
    """Build the per-core Bass graph. Identical on all 8 cores (SPMD);
    only the DMA'd data differs. bounds[g] = (row_start, row_end) of
    group g in the sorted sample order."""
    import concourse.bacc as bacc
    import concourse.mybir as mybir
    from concourse import tile

    bf = mybir.dt.bfloat16
    f32 = mybir.dt.float32

    nc = bacc.Bacc(None, target_bir_lowering=False)
    xt_d = nc.declare_dram_parameter("xt", [128, IN], bf, isOutput=False)
    wq_d = nc.declare_dram_parameter("wq", [n_q, 128, 4 * OSH * KT], bf, isOutput=False)
    ind_d = nc.declare_dram_parameter("ind", [4, n_q * 128], bf, isOutput=False)
    brhs_d = nc.declare_dram_parameter("brhs", [4, n_q * 4 * OSH], bf, isOutput=False)
    out_d = nc.declare_dram_parameter("out", [128, OSH], f32, isOutput=True)

    FD = 4 * OSH  # 512: matmul moving free dim (4 group-column-blocks)

    with tile.TileContext(nc) as tc:
        with (
            tc.tile_pool(name="const", bufs=1) as cpool,
            tc.tile_pool(name="w", bufs=3) as wpool,
            tc.tile_pool(name="q", bufs=3) as qpool,
            tc.tile_pool(name="ps", bufs=4, space="PSUM") as pspool,
        ):
            xt_sb = cpool.tile([128, IN], bf)
            nc.sync.dma_start(xt_sb[:], xt_d[:])
            ind_sb = cpool.tile([4, n_q * 128], bf)
            nc.sync.dma_start(ind_sb[:], ind_d[:])
            brhs_sb = cpool.tile([4, n_q * FD], bf)
            nc.sync.dma_start(brhs_sb[:], brhs_d[:])

            for q in range(n_q):
                w_sb = wpool.tile([128, KT * FD], bf)
                nc.sync.dma_start(w_sb[:], wq_d[q])
                ps = pspool.tile([128, FD], f32)
                for k in range(KT):
                    nc.tensor.matmul(
                        ps[:],
                        xt_sb[:, k * 128 : (k + 1) * 128],
                        w_sb[:, k * FD : (k + 1) * FD],
                        start=(k == 0),
                        stop=False,
                    )
                nc.tensor.matmul(
                    ps[:],
                    ind_sb[:, q * 128 : (q + 1) * 128],
                    brhs_sb[:, q * FD : (q + 1) * FD],
                    start=False,
                    stop=True,
                )
                # Compute engines need partition offsets that are multiples
                # of 32, so evict the whole quad PSUM to SBUF, then let DMA
                # (no partition alignment rules) extract each group's rows
                # from its own 128-column block.
                cp = qpool.tile([128, FD], f32)
                nc.vector.tensor_copy(cp[:], ps[:])
                for g in range(4):
                    gi = 4 * q + g
                    if gi >= len(bounds):
                        continue
                    s, e = bounds[gi]
                    if e > s:
                        nc.scalar.dma_start(
                            out_d[s:e, :], cp[s:e, g * OSH : (g + 1) * OSH]
                        )

    nc.finalize()
    return nc


def kernel(X, W, bias, Werr, Berr, loc_id):
    global LAST_EXEC_TIME_NS
    _install_ntff_hook()
    from concourse.bass_utils import run_bass_kernel_spmd

    X = np.asarray(X, dtype=np.float32)
    W = np.asarray(W, dtype=np.float32)
    bias = np.asarray(bias, dtype=np.float32)
    Werr = np.asarray(Werr, dtype=np.float32)
    Berr = np.asarray(Berr, dtype=np.float32)
    loc_id = np.asarray(loc_id)

    # ---- host-side dedup / grouping -------------------------------------
    U, inv = np.unique(loc_id, return_inverse=True)
    n_u = len(U)
    order = np.argsort(inv, kind="stable")
    inv_sorted = inv[order]
    n_q = (n_u + 3) // 4
    n_gp = 4 * n_q

    counts = np.bincount(inv_sorted, minlength=n_gp)
    ends = np.cumsum(counts)
    starts = ends - counts
    bounds = tuple((int(starts[g]), int(ends[g])) for g in range(n_u))

    # ---- host-side packing ----------------------------------------------
    # Compacted pool with W folded in, padded to a multiple of 4 groups.
    A = np.zeros((n_gp, IN, OUT), dtype=np.float32)
    A[:n_u] = Werr[U]
    A[:n_u] *= W
    B = A.astype(BF16)
    # [q, g, k, p, core, o] -> [core, q, p, k, g, o]
    B = B.reshape(n_q, 4, KT, 128, N_CORES, OSH).transpose(4, 0, 3, 2, 1, 5)
    wq_percore = np.ascontiguousarray(B).reshape(N_CORES, n_q, 128, KT * 4 * OSH)

    # X^T in k-major-per-partition layout: xt[p, k, b] = X_sorted[b, 128k+p]
    Xs = X[order].astype(BF16)
    xt = np.ascontiguousarray(Xs.T.reshape(KT, 128, 128).transpose(1, 0, 2)).reshape(
        128, IN
    )

    # Group indicator (stationary of the bias matmul): ind[g, q, b]
    ind = np.zeros((4, n_q, 128), dtype=BF16)
    q_of = inv_sorted // 4
    g_of = inv_sorted % 4
    ind[g_of, q_of, np.arange(BATCH)] = 1.0
    ind = ind.reshape(4, n_q * 128)

    # Bias moving operand: per group one bias row, in its own column block.
    membias = Berr[U] * bias  # [n_u, OUT]
    brhs = np.zeros((4, n_q, 4, OUT), dtype=np.float32)
    for gi in range(n_u):
        q, g = divmod(gi, 4)
        brhs[g, q, g] = membias[gi]
    brhs = brhs.astype(BF16)
    # per-core column slice -> [4, n_q, 4, OSH] -> [4, n_q*4*OSH]
    brhs_percore = [
        np.ascontiguousarray(brhs[:, :, :, c * OSH : (c + 1) * OSH]).reshape(
            4, n_q * 4 * OSH
        )
        for c in range(N_CORES)
    ]

    # ---- build / fetch compiled graph -----------------------------------
    key = (n_q, bounds)
    nc = _NC_CACHE.get(key)
    if nc is None:
        nc = _build_graph(n_q, bounds)
        _NC_CACHE[key] = nc

    in_maps = [
        {
            "xt": xt,
            "wq": wq_percore[c],
            "ind": ind,
            "brhs": brhs_percore[c],
        }
        for c in range(N_CORES)
    ]

    trace = bool(os.environ.get("BASS_TRACE"))
    res = run_bass_kernel_spmd(nc, in_maps, core_ids=list(range(N_CORES)), trace=trace)
    LAST_EXEC_TIME_NS = res.exec_time_ns

    Zs = np.concatenate([res.results[c]["out"] for c in range(N_CORES)], axis=1)
    Z = np.empty((BATCH, OUT), dtype=np.float32)
    Z[order] = Zs
    return Z


# revision 7
# speedup vs baseline: 1.3458x; 1.3458x over previous
"""AConnect forward kernel for one TRN2 chip (8 NeuronCores).

Computes Z[b] = X[b] @ (W * Werr[loc_id[b]]) + Berr[loc_id[b]] * bias
for B=128, IN=OUT=1024, POOL=200.

Strategy (data-parallel over the OUT dim, dedup over the pool ids):
  - Host: dedup loc_id into n_u unique pool entries (~95 of 128 draws),
    fold W into the gathered pool (bf16), sort samples by group, and pack
    per-core weight slabs so every Werr byte is read exactly once
    chip-wide. Each core owns a 128-column slice of OUT.
  - Device: for each "quad" of 4 groups, one 1 MiB DMA streams the packed
    weights; TensorE runs 8 k-tile matmuls (stationary = X^T tiles,
    moving = packed weights, N=512) plus one rank-4 indicator matmul that
    adds the per-group bias row; DVE extracts each group's 128-column
    block of PSUM into the output tile.
"""

import os
import sys
import types

import numpy as np

if "/opt/trn_rl_repo" not in sys.path:
    sys.path.insert(0, "/opt/trn_rl_repo")

import ml_dtypes

BF16 = ml_dtypes.bfloat16

BATCH, IN, OUT, POOL = 128, 1024, 1024, 200
N_CORES = 8
OSH = OUT // N_CORES  # 128 output columns per core
KT = IN // 128        # 8 k-tiles


def _install_ntff_hook():
    """Make run_bass_kernel_spmd(trace=True) work under axon: the glue
    module antenv.axon_hooks is absent from this image, so inject it."""
    if "antenv.axon_hooks" in sys.modules:
        return
    try:
        from trn_agent_boot.trn_boot import _ntff_profile_via_ctypes

        hook = _ntff_profile_via_ctypes("/opt/axon/libaxon_pjrt.so")
    except Exception:
        hook = None
    mod = types.ModuleType("antenv.axon_hooks")
    mod.get_axon_ntff_profile_hook = lambda: hook
    mod.set_axon_ntff_profile_hook = lambda h: None
    sys.modules["antenv.axon_hooks"] = mod


_NC_CACHE: dict = {}
LAST_EXEC_TIME_NS = None


def _build_graph(n_q, bounds):
    """Build the per-core Bass graph. Identical on all 8 cores (SPMD);
    only the DMA'd data differs. bounds[g] = (row_start, row_end) of
    group g in the sorted sample order."""
    import concourse.bacc as bacc
    import concourse.mybir as mybir
    from concourse import tile

    bf = mybir.dt.bfloat16
    f32 = mybir.dt.float32

    nc = bacc.Bacc(None, target_bir_lowering=False)
    xt_d = nc.declare_dram_parameter("xt", [128, IN], bf, isOutput=False)
    wq_d = nc.declare_dram_parameter("wq", [n_q, 128, 4 * OSH * KT], bf, isOutput=False)
    ind_d = nc.declare_dram_parameter("ind", [4, n_q * 128], bf, isOutput=False)
    brhs_d = nc.declare_dram_parameter("brhs", [4, n_q * 4 * OSH], bf, isOutput=False)
    masks_d = nc.declare_dram_parameter("masks", [128, 4 * n_q], f32, isOutput=False)
    out_d = nc.declare_dram_parameter("out", [128, OSH], f32, isOutput=True)

    FD = 4 * OSH  # 512: matmul moving free dim (4 group-column-blocks)

    with tile.TileContext(nc) as tc:
        with (
            tc.tile_pool(name="const", bufs=1) as cpool,
            tc.tile_pool(name="w", bufs=6) as wpool,
            tc.tile_pool(name="tmp", bufs=6) as tpool,
            tc.tile_pool(name="ps", bufs=6, space="PSUM") as pspool,
        ):
            xt_sb = cpool.tile([128, IN], bf)
            nc.sync.dma_start(xt_sb[:], xt_d[:])
            ind_sb = cpool.tile([4, n_q * 128], bf)
            nc.sync.dma_start(ind_sb[:], ind_d[:])
            brhs_sb = cpool.tile([4, n_q * FD], bf)
            nc.sync.dma_start(brhs_sb[:], brhs_d[:])
            masks_sb = cpool.tile([128, 4 * n_q], f32)
            nc.sync.dma_start(masks_sb[:], masks_d[:])
            out_sb = cpool.tile([128, OSH], f32)
            nc.vector.memset(out_sb[:], 0.0)

            for q in range(n_q):
                w_sb = wpool.tile([128, KT * FD], bf)
                nc.sync.dma_start(w_sb[:], wq_d[q])
                ps = pspool.tile([128, FD], f32)
                for k in range(KT):
                    nc.tensor.matmul(
                        ps[:],
                        xt_sb[:, k * 128 : (k + 1) * 128],
                        w_sb[:, k * FD : (k + 1) * FD],
                        start=(k == 0),
                        stop=False,
                    )
                nc.tensor.matmul(
                    ps[:],
                    ind_sb[:, q * 128 : (q + 1) * 128],
                    brhs_sb[:, q * FD : (q + 1) * FD],
                    start=False,
                    stop=True,
                )
                # Compute engines need partition offsets that are multiples
                # of 32, so extract each group's rows with a full-partition
                # per-partition-mask multiply, then accumulate. Rows outside
                # the group contribute zero.
                for g in range(4):
                    gi = 4 * q + g
                    if gi >= len(bounds):
                        continue
                    s, e = bounds[gi]
                    if e > s:
                        tmp = tpool.tile([128, OSH], f32)
                        nc.vector.tensor_scalar_mul(
                            tmp[:],
                            ps[:, g * OSH : (g + 1) * OSH],
                            masks_sb[:, gi : gi + 1],
                        )
                        nc.vector.tensor_add(out_sb[:], out_sb[:], tmp[:])

            nc.sync.dma_start(out_d[:], out_sb[:])

    nc.finalize()
    return nc


def kernel(X, W, bias, Werr, Berr, loc_id):
    global LAST_EXEC_TIME_NS
    _install_ntff_hook()
    from concourse.bass_utils import run_bass_kernel_spmd

    X = np.asarray(X, dtype=np.float32)
    W = np.asarray(W, dtype=np.float32)
    bias = np.asarray(bias, dtype=np.float32)
    Werr = np.asarray(Werr, dtype=np.float32)
    Berr = np.asarray(Berr, dtype=np.float32)
    loc_id = np.asarray(loc_id)

    # ---- host-side dedup / grouping -------------------------------------
    U, inv = np.unique(loc_id, return_inverse=True)
    n_u = len(U)
    order = np.argsort(inv, kind="stable")
    inv_sorted = inv[order]
    n_q = (n_u + 3) // 4
    n_gp = 4 * n_q

    counts = np.bincount(inv_sorted, minlength=n_gp)
    ends = np.cumsum(counts)
    starts = ends - counts
    bounds = tuple((int(starts[g]), int(ends[g])) for g in range(n_u))

    # ---- host-side packing ----------------------------------------------
    # Compacted pool with W folded in, padded to a multiple of 4 groups.
    A = np.zeros((n_gp, IN, OUT), dtype=np.float32)
    A[:n_u] = Werr[U]
    A[:n_u] *= W
    B = A.astype(BF16)
    # [q, g, k, p, core, o] -> [core, q, p, k, g, o]
    B = B.reshape(n_q, 4, KT, 128, N_CORES, OSH).transpose(4, 0, 3, 2, 1, 5)
    wq_percore = np.ascontiguousarray(B).reshape(N_CORES, n_q, 128, KT * 4 * OSH)

    # X^T in k-major-per-partition layout: xt[p, k, b] = X_sorted[b, 128k+p]
    Xs = X[order].astype(BF16)
    xt = np.ascontiguousarray(Xs.T.reshape(KT, 128, 128).transpose(1, 0, 2)).reshape(
        128, IN
    )

    # Group indicator (stationary of the bias matmul): ind[g, q, b]
    ind = np.zeros((4, n_q, 128), dtype=BF16)
    q_of = inv_sorted // 4
    g_of = inv_sorted % 4
    ind[g_of, q_of, np.arange(BATCH)] = 1.0
    ind = ind.reshape(4, n_q * 128)

    # Per-partition row masks for the epilogue: masks[b, gi] = 1 iff sample
    # b (sorted order) belongs to group gi.
    masks = np.zeros((128, 4 * n_q), dtype=np.float32)
    masks[np.arange(BATCH), inv_sorted] = 1.0

    # Bias moving operand: per group one bias row, in its own column block.
    membias = Berr[U] * bias  # [n_u, OUT]
    brhs = np.zeros((4, n_q, 4, OUT), dtype=np.float32)
    for gi in range(n_u):
        q, g = divmod(gi, 4)
        brhs[g, q, g] = membias[gi]
    brhs = brhs.astype(BF16)
    # per-core column slice -> [4, n_q, 4, OSH] -> [4, n_q*4*OSH]
    brhs_percore = [
        np.ascontiguousarray(brhs[:, :, :, c * OSH : (c + 1) * OSH]).reshape(
            4, n_q * 4 * OSH
        )
        for c in range(N_CORES)
    ]

    # ---- build / fetch compiled graph -----------------------------------
    key = (n_q, bounds)
    nc = _NC_CACHE.get(key)
    if nc is None:
        nc = _build_graph(n_q, bounds)
        _NC_CACHE[key] = nc

    in_maps = [
        {
            "xt": xt,
            "wq": wq_percore[c],
            "ind": ind,
            "brhs": brhs_percore[c],
            "masks": masks,
        }
        for c in range(N_CORES)
    ]

    trace = bool(os.environ.get("BASS_TRACE"))
    res = run_bass_kernel_spmd(nc, in_maps, core_ids=list(range(N_CORES)), trace=trace)
    LAST_EXEC_TIME_NS = res.exec_time_ns

    Zs = np.concatenate([res.results[c]["out"] for c in range(N_CORES)], axis=1)
    Z = np.empty((BATCH, OUT), dtype=np.float32)
    Z[order] = Zs
    return Z


# revision 9
# speedup vs baseline: 1.4428x; 1.0720x over previous
"""AConnect forward kernel for one TRN2 chip (8 NeuronCores).

Computes Z[b] = X[b] @ (W * Werr[loc_id[b]]) + Berr[loc_id[b]] * bias
for B=128, IN=OUT=1024, POOL=200.

Strategy (data-parallel over the OUT dim, dedup over the pool ids):
  - Host: dedup loc_id into n_u unique pool entries (~95 of 128 draws),
    fold W into the gathered pool (bf16), sort samples by group, and pack
    per-core weight slabs so every Werr byte is read exactly once
    chip-wide. Each core owns a 128-column slice of OUT.
  - Device: for each "quad" of 4 groups, one 1 MiB DMA streams the packed
    weights; TensorE runs 8 k-tile matmuls (stationary = X^T tiles,
    moving = packed weights, N=512) plus one rank-4 indicator matmul that
    adds the per-group bias row; DVE extracts each group's 128-column
    block of PSUM into the output tile.
"""

import os
import sys
import types

import numpy as np

if "/opt/trn_rl_repo" not in sys.path:
    sys.path.insert(0, "/opt/trn_rl_repo")

import ml_dtypes

BF16 = ml_dtypes.bfloat16

BATCH, IN, OUT, POOL = 128, 1024, 1024, 200
N_CORES = 8
OSH = OUT // N_CORES  # 128 output columns per core
KT = IN // 128        # 8 k-tiles


def _install_ntff_hook():
    """Make run_bass_kernel_spmd(trace=True) work under axon: the glue
    module antenv.axon_hooks is absent from this image, so inject it."""
    if "antenv.axon_hooks" in sys.modules:
        return
    try:
        from trn_agent_boot.trn_boot import _ntff_profile_via_ctypes

        hook = _ntff_profile_via_ctypes("/opt/axon/libaxon_pjrt.so")
    except Exception:
        hook = None
    mod = types.ModuleType("antenv.axon_hooks")
    mod.get_axon_ntff_profile_hook = lambda: hook
    mod.set_axon_ntff_profile_hook = lambda h: None
    sys.modules["antenv.axon_hooks"] = mod


_NC_CACHE: dict = {}
LAST_EXEC_TIME_NS = None


def _build_graph(n_q, bounds):
    """Build the per-core Bass graph. Identical on all 8 cores (SPMD);
    only the DMA'd data differs. bounds[g] = (row_start, row_end) of
    group g in the sorted sample order."""
    import concourse.bacc as bacc
    import concourse.mybir as mybir
    from concourse import tile

    bf = mybir.dt.bfloat16
    f32 = mybir.dt.float32

    nc = bacc.Bacc(None, target_bir_lowering=False)
    xt_d = nc.declare_dram_parameter("xt", [128, IN], bf, isOutput=False)
    wq_d = nc.declare_dram_parameter("wq", [n_q, 128, 4 * OSH * KT], bf, isOutput=False)
    ind_d = nc.declare_dram_parameter("ind", [4, n_q * 128], bf, isOutput=False)
    brhs_d = nc.declare_dram_parameter("brhs", [4, n_q * 4 * OSH], bf, isOutput=False)
    masks_d = nc.declare_dram_parameter("masks", [128, 4 * n_q], f32, isOutput=False)
    out_d = nc.declare_dram_parameter("out", [128, OSH], f32, isOutput=True)

    FD = 4 * OSH  # 512: matmul moving free dim (4 group-column-blocks)

    with tile.TileContext(nc) as tc:
        with (
            tc.tile_pool(name="const", bufs=1) as cpool,
            tc.tile_pool(name="w", bufs=8) as wpool,
            tc.tile_pool(name="tmp", bufs=4) as tpool,
            tc.tile_pool(name="red", bufs=4) as rpool,
            tc.tile_pool(name="ps", bufs=6, space="PSUM") as pspool,
        ):
            xt_sb = cpool.tile([128, IN], bf)
            nc.sync.dma_start(xt_sb[:], xt_d[:])
            ind_sb = cpool.tile([4, n_q * 128], bf)
            nc.sync.dma_start(ind_sb[:], ind_d[:])
            brhs_sb = cpool.tile([4, n_q * FD], bf)
            nc.sync.dma_start(brhs_sb[:], brhs_d[:])
            masks_sb = cpool.tile([128, 4 * n_q], f32)
            nc.sync.dma_start(masks_sb[:], masks_d[:])
            out_sb = cpool.tile([128, OSH], f32)
            nc.vector.memset(out_sb[:], 0.0)

            for q in range(n_q):
                w_sb = wpool.tile([128, KT * FD], bf)
                dma_eng = nc.sync if q % 2 == 0 else nc.scalar
                dma_eng.dma_start(w_sb[:], wq_d[q])
                ps = pspool.tile([128, FD], f32)
                for k in range(KT):
                    nc.tensor.matmul(
                        ps[:],
                        xt_sb[:, k * 128 : (k + 1) * 128],
                        w_sb[:, k * FD : (k + 1) * FD],
                        start=(k == 0),
                        stop=False,
                    )
                nc.tensor.matmul(
                    ps[:],
                    ind_sb[:, q * 128 : (q + 1) * 128],
                    brhs_sb[:, q * FD : (q + 1) * FD],
                    start=False,
                    stop=True,
                )
                # Row extraction (compute engines need 32-aligned partition
                # offsets, so work on full partitions): multiply the quad's
                # PSUM by the per-(row, group) mask broadcast along the
                # output columns, reduce over the 4 group blocks, then
                # accumulate. Rows outside the quad's groups contribute 0.
                tmp4 = tpool.tile([128, FD], f32)
                mk = masks_sb[:, 4 * q : 4 * q + 4]
                mk_bc = mk.copy()
                mk_bc.ap = mk.ap[:-1] + [mk.ap[-1], [0, OSH]]
                nc.vector.tensor_mul(
                    tmp4[:].rearrange("p (g o) -> p g o", g=4),
                    ps[:].rearrange("p (g o) -> p g o", g=4),
                    mk_bc,
                )
                red = rpool.tile([128, OSH], f32)
                nc.vector.reduce_sum(
                    red[:],
                    tmp4[:].rearrange("p (g o) -> p o g", g=4),
                    axis=mybir.AxisListType.X,
                )
                nc.vector.tensor_add(out_sb[:], out_sb[:], red[:])

            nc.sync.dma_start(out_d[:], out_sb[:])

    nc.finalize()
    return nc


def kernel(X, W, bias, Werr, Berr, loc_id):
    global LAST_EXEC_TIME_NS
    _install_ntff_hook()
    from concourse.bass_utils import run_bass_kernel_spmd

    X = np.asarray(X, dtype=np.float32)
    W = np.asarray(W, dtype=np.float32)
    bias = np.asarray(bias, dtype=np.float32)
    Werr = np.asarray(Werr, dtype=np.float32)
    Berr = np.asarray(Berr, dtype=np.float32)
    loc_id = np.asarray(loc_id)

    # ---- host-side dedup / grouping -------------------------------------
    U, inv = np.unique(loc_id, return_inverse=True)
    n_u = len(U)
    order = np.argsort(inv, kind="stable")
    inv_sorted = inv[order]
    n_q = (n_u + 3) // 4
    n_gp = 4 * n_q

    counts = np.bincount(inv_sorted, minlength=n_gp)
    ends = np.cumsum(counts)
    starts = ends - counts
    bounds = tuple((int(starts[g]), int(ends[g])) for g in range(n_u))

    # ---- host-side packing ----------------------------------------------
    # Compacted pool with W folded in, padded to a multiple of 4 groups.
    A = np.zeros((n_gp, IN, OUT), dtype=np.float32)
    A[:n_u] = Werr[U]
    A[:n_u] *= W
    B = A.astype(BF16)
    # [q, g, k, p, core, o] -> [core, q, p, k, g, o]
    B = B.reshape(n_q, 4, KT, 128, N_CORES, OSH).transpose(4, 0, 3, 2, 1, 5)
    wq_percore = np.ascontiguousarray(B).reshape(N_CORES, n_q, 128, KT * 4 * OSH)

    # X^T in k-major-per-partition layout: xt[p, k, b] = X_sorted[b, 128k+p]
    Xs = X[order].astype(BF16)
    xt = np.ascontiguousarray(Xs.T.reshape(KT, 128, 128).transpose(1, 0, 2)).reshape(
        128, IN
    )

    # Group indicator (stationary of the bias matmul): ind[g, q, b]
    ind = np.zeros((4, n_q, 128), dtype=BF16)
    q_of = inv_sorted // 4
    g_of = inv_sorted % 4
    ind[g_of, q_of, np.arange(BATCH)] = 1.0
    ind = ind.reshape(4, n_q * 128)

    # Per-partition row masks for the epilogue: masks[b, gi] = 1 iff sample
    # b (sorted order) belongs to group gi.
    masks = np.zeros((128, 4 * n_q), dtype=np.float32)
    masks[np.arange(BATCH), inv_sorted] = 1.0

    # Bias moving operand: per group one bias row, in its own column block.
    membias = Berr[U] * bias  # [n_u, OUT]
    brhs = np.zeros((4, n_q, 4, OUT), dtype=np.float32)
    for gi in range(n_u):
        q, g = divmod(gi, 4)
        brhs[g, q, g] = membias[gi]
    brhs = brhs.astype(BF16)
    # per-core column slice -> [4, n_q, 4, OSH] -> [4, n_q*4*OSH]
    brhs_percore = [
        np.ascontiguousarray(brhs[:, :, :, c * OSH : (c + 1) * OSH]).reshape(
            4, n_q * 4 * OSH
        )
        for c in range(N_CORES)
    ]

    # ---- build / fetch compiled graph -----------------------------------
    key = (n_q, bounds)
    nc = _NC_CACHE.get(key)
    if nc is None:
        nc = _build_graph(n_q, bounds)
        _NC_CACHE[key] = nc

    in_maps = [
        {
            "xt": xt,
            "wq": wq_percore[c],
            "ind": ind,
            "brhs": brhs_percore[c],
            "masks": masks,
        }
        for c in range(N_CORES)
    ]

    trace = bool(os.environ.get("BASS_TRACE"))
    res = run_bass_kernel_spmd(nc, in_maps, core_ids=list(range(N_CORES)), trace=trace)
    LAST_EXEC_TIME_NS = res.exec_time_ns

    Zs = np.concatenate([res.results[c]["out"] for c in range(N_CORES)], axis=1)
    Z = np.empty((BATCH, OUT), dtype=np.float32)
    Z[order] = Zs
    return Z
